# revision 1
# baseline (speedup 1.0000x reference)
"""Trainium2 Bass kernel for nn_AbstractModel_70394513981830 (spiking network).

Per step: s' = sigmoid(segment_sum(W * s[src], dst) - 1), 300 sequential steps,
last 200 recorded. 8 NeuronCores, dst-sharded (NC k owns dsts [12500k,12500k+12500)).

Single fused kernel launch runs all 300 steps (unrolled; one straight-line
AllGather per step). Per-step pipeline on each NC (its 800k edges):

1. s_all [128, 848] bf16 holds the full state: node n = 12500*k + d lives at
   partition d%128, column 106*k + d//128; columns 98..105 of each sender block
   are extension copies of "overloaded" nodes (see below).
2. Expansion multiply (DVE): per-edge products P1[p, occ*848 + j] =
   W_exp[p, occ*848 + j] * s_all[p, j] via a stride-0 broadcast view. Each
   node-column j has a run of R=16 occurrence slots; a (node, receiver) pair
   with >16 in-edges spills into the node's extension column (16 more).
3. Route each product to the partition owning its dst in 3 stages:
   a. local_scatter (per-partition idxs) P1 -> R1 at position 128*b + p_dst
      (b = per (src-partition, dst-partition) bucket fill), in windows of 2047.
   b. full 128x128 block transposes of R1: DVE stream_transpose (32x32) + 16
      block-grid DMAs => R2[p_dst, 128*b + q_src] = R1[q_src, 128*b + p_dst].
   c. local_scatter R2 -> R3: per-partition dst runs, padded to multiples of 8.
4. reduce8 (f32) -> chunk sums; bf16; local_scatter chunks into per-dst
   16-slot bins; reduce16 -> per-dst sums [128, 98] f32; sigmoid(x-1) -> bf16.
5. local_scatter fills the 8 extension columns; shard [128, 106] -> DRAM;
   AllGather; 8 DMAs rebuild s_all; raster row written per step.
"""

import numpy as np

LAST_RUN_NS = 0

N_NODES = 100_000
N_EDGES = 6_400_000
THETA = 1.0
NC = 8
NCD = N_NODES // NC          # 12500 dsts / nodes per NC
COLS = 98                    # ceil(12500 / 128) node columns per NC
EXT = 12                     # extension columns per NC
CPN = COLS + EXT             # 106 columns per NC block in s_all
NODECOLS = NC * CPN          # 848
R = 16                       # occurrence slots per node column
M1 = R * NODECOLS            # 13568 expansion slots per partition
WIN = 2046                   # local_scatter out window size (even)
BINS = 16                    # chunk bins per dst
MAXRANK = 98                 # dst ranks per partition


def _group_rank(keys):
    """Rank of each element within its equal-key group (keys int64 1-D)."""
    order = np.argsort(keys, kind="stable")
    sk = keys[order]
    first = np.r_[0, np.flatnonzero(np.diff(sk)) + 1]
    grp_start = np.zeros(len(sk), dtype=np.int64)
    grp_start[first] = first
    np.maximum.accumulate(grp_start, out=grp_start)
    rank_sorted = np.arange(len(sk)) - grp_start
    rank = np.empty(len(sk), dtype=np.int64)
    rank[order] = rank_sorted
    return rank


def _preprocess(x, W, edge_index):
    import ml_dtypes

    src = np.asarray(edge_index[0], dtype=np.int64)
    dst = np.asarray(edge_index[1], dtype=np.int64)
    W = np.asarray(W, dtype=np.float32)
    s0 = np.asarray(x, dtype=np.float32).reshape(-1)

    k_r = dst // NCD                       # receiver NC per edge
    ks = src // NCD                        # sender NC per src node
    ds = src % NCD
    q_s = ds % 128                         # src partition
    base_j = CPN * ks + ds // 128          # base node column

    # (node, receiver) in-multiplicity; extension column assignment (global)
    cnt = np.bincount(src * NC + k_r, minlength=N_NODES * NC).reshape(N_NODES, NC)
    assert cnt.max() <= 2 * R, f"node multiplicity {cnt.max()} > {2*R}"
    over = (cnt > R).any(axis=1)           # nodes needing an extension column
    node_ks = np.arange(N_NODES) // NCD
    node_q = (np.arange(N_NODES) % NCD) % 128
    ext_col = np.full(N_NODES, -1, dtype=np.int64)
    ext_slot = np.full(N_NODES, -1, dtype=np.int64)
    ov_idx = np.flatnonzero(over)
    slot = _group_rank(node_ks[ov_idx] * 128 + node_q[ov_idx])
    assert len(slot) == 0 or slot.max() < EXT, \
        f"extension slots per (sender,partition) exceed {EXT}: {slot.max()+1}"
    ext_slot[ov_idx] = slot
    ext_col[ov_idx] = CPN * node_ks[ov_idx] + COLS + slot

    per_core = []
    B_all, L3_all = [], []
    cores_tmp = []
    for k in range(NC):
        sel = np.flatnonzero(k_r == k)
        e_src = src[sel]
        e_w = W[sel]
        dl = dst[sel] - NCD * k
        p_d = dl % 128
        r_d = dl // 128
        eq = q_s[sel]

        occ = _group_rank(e_src)
        use_ext = occ >= R
        j = np.where(use_ext, ext_col[e_src], base_j[sel])
        occ_slot = np.where(use_ext, occ - R, occ)
        assert (j >= 0).all() and (occ_slot < R).all()
        m1 = occ_slot * NODECOLS + j       # P1 slot (partition eq)

        b = _group_rank(eq * 128 + p_d)    # bucket fill (q_src, p_dst)
        B = int(b.max()) + 1

        # dst runs (per dst-partition), padded to multiples of 8
        deg = np.bincount(p_d * MAXRANK + r_d,
                          minlength=128 * MAXRANK).reshape(128, MAXRANK)
        runlen = (deg + 7) // 8 * 8
        starts = np.zeros_like(runlen)
        starts[:, 1:] = np.cumsum(runlen, axis=1)[:, :-1]
        L3 = int((starts[:, -1] + runlen[:, -1]).max())
        within = _group_rank(p_d * MAXRANK + r_d)
        slot3 = starts[p_d, r_d] + within

        cores_tmp.append(dict(
            e_src=e_src, e_w=e_w, p_d=p_d, r_d=r_d, eq=eq, m1=m1, b=b,
            slot3=slot3, deg=deg, runlen=runlen, starts=starts,
        ))
        B_all.append(B)
        L3_all.append(L3)

    B = max(B_all)
    M2 = 128 * B                           # routed-layout size
    W1 = -(-M2 // WIN)                     # stage-a windows
    L3 = -(-max(L3_all) // 16) * 16        # multiple of 16 (even NCHUNK)
    W3 = -(-L3 // WIN)
    NCHUNK = L3 // 8
    assert M1 % 2 == 0 and M2 % 2 == 0 and NCHUNK % 2 == 0

    for k in range(NC):
        t = cores_tmp[k]
        eq, m1, p_d, b = t["eq"], t["m1"], t["p_d"], t["b"]
        r1pos = 128 * b + p_d
        idx1 = np.full((W1, 128, M1), -1, dtype=np.int16)
        w = r1pos // WIN
        idx1[w, eq, m1] = (r1pos - w * WIN).astype(np.int16)

        m2 = 128 * b + eq                  # position after transpose (part p_d)
        slot3 = t["slot3"]
        idx3 = np.full((W3, 128, M2), -1, dtype=np.int16)
        w = slot3 // WIN
        idx3[w, p_d, m2] = (slot3 - w * WIN).astype(np.int16)

        # chunk -> bin mapping
        runlen, starts = t["runlen"], t["starts"]
        idx_bins = np.full((128, NCHUNK), -1, dtype=np.int16)
        for p in range(128):
            for r in range(MAXRANK):
                rl = runlen[p, r]
                if rl == 0:
                    continue
                c0 = starts[p, r] // 8
                nch = rl // 8
                assert nch <= BINS
                idx_bins[p, c0:c0 + nch] = np.arange(
                    BINS * r, BINS * r + nch, dtype=np.int16)

        w_exp = np.zeros((128, M1), dtype=ml_dtypes.bfloat16)
        w_exp[eq, m1] = t["e_w"].astype(ml_dtypes.bfloat16)

        # sender-side extension fill: this core's own overloaded nodes
        extf = np.full((128, COLS), -1, dtype=np.int16)
        own = np.arange(NCD * k, NCD * (k + 1))
        ov = over[own]
        d_own = np.arange(NCD)[ov]
        extf[d_own % 128, d_own // 128] = ext_slot[own[ov]].astype(np.int16)

        per_core.append(dict(w_exp=w_exp, idx1=idx1, idx3=idx3,
                             idx_bins=idx_bins, extf=extf))

    # initial s_all (same for all cores)
    s_pad = np.zeros(128 * NODECOLS, dtype=np.float32).reshape(128, NODECOLS)
    n = np.arange(N_NODES)
    s_pad[(n % NCD) % 128, CPN * (n // NCD) + (n % NCD) // 128] = s0
    s_pad[node_q[ov_idx], ext_col[ov_idx]] = s0[ov_idx]
    s_all0 = s_pad.astype(ml_dtypes.bfloat16)

    meta = dict(B=B, M2=M2, W1=W1, W3=W3, L3=L3, NCHUNK=NCHUNK, s_all0=s_all0)
    return meta, per_core


def _np_step(meta, per_core, s_all):
    """Numpy simulation of one device step. s_all [128, 848] bf16 (all cores
    share it). Returns (new s_all bf16, shards list of [128, COLS] bf16)."""
    import ml_dtypes

    B, M2, W1, W3 = meta["B"], meta["M2"], meta["W1"], meta["W3"]
    L3, NCHUNK = meta["L3"], meta["NCHUNK"]
    shards = []
    for k in range(NC):
        pc = per_core[k]
        s_view = np.tile(s_all[:, None, :], (1, R, 1)).reshape(128, M1)
        p1 = (pc["w_exp"].astype(np.float32) * s_view.astype(np.float32)
              ).astype(ml_dtypes.bfloat16)
        r1 = np.zeros((128, W1 * WIN), dtype=ml_dtypes.bfloat16)
        for w in range(W1):
            ii = pc["idx1"][w]
            pmask, mmask = np.nonzero(ii >= 0)
            r1[pmask, w * WIN + ii[pmask, mmask]] = p1[pmask, mmask]
        r2 = np.zeros((128, M2), dtype=ml_dtypes.bfloat16)
        blk = r1[:, :M2].reshape(128, B, 128)
        r2 = blk.transpose(2, 1, 0).reshape(128, M2)
        r3 = np.zeros((128, L3), dtype=ml_dtypes.bfloat16)
        for w in range(W3):
            ii = pc["idx3"][w]
            pmask, mmask = np.nonzero(ii >= 0)
            r3[pmask, w * WIN + ii[pmask, mmask]] = r2[pmask, mmask]
        chunks = r3.astype(np.float32).reshape(128, NCHUNK, 8).sum(-1)
        chunks_bf = chunks.astype(ml_dtypes.bfloat16)
        bins = np.zeros((128, MAXRANK * BINS), dtype=ml_dtypes.bfloat16)
        ib = pc["idx_bins"]
        pmask, cmask = np.nonzero(ib >= 0)
        bins[pmask, ib[pmask, cmask]] = chunks_bf[pmask, cmask]
        sums = bins.astype(np.float32).reshape(128, MAXRANK, BINS).sum(-1)
        snew = (1.0 / (1.0 + np.exp(-(sums - THETA)))).astype(ml_dtypes.bfloat16)
        shards.append(snew)

    new_sall = np.zeros_like(s_all)
    for k in range(NC):
        pc = per_core[k]
        blkk = np.zeros((128, CPN), dtype=ml_dtypes.bfloat16)
        blkk[:, :COLS] = shards[k]
        ef = pc["extf"]
        pmask, cmask = np.nonzero(ef >= 0)
        blkk[pmask, COLS + ef[pmask, cmask]] = shards[k][pmask, cmask]
        new_sall[:, CPN * k:CPN * (k + 1)] = blkk
    return new_sall, shards


def _sall_to_s(s_all):
    """Extract the flat [N_NODES] f32 state from an s_all table."""
    n = np.arange(N_NODES)
    return s_all[(n % NCD) % 128,
                 CPN * (n // NCD) + (n % NCD) // 128].astype(np.float32)


def _build(meta, steps):
    """Build the unrolled bass program (shared by all 8 cores)."""
    import concourse.bass as bass
    import concourse.bacc as bacc
    import concourse.mybir as mybir
    import concourse.tile as tile

    f32 = mybir.dt.float32
    bf16 = mybir.dt.bfloat16
    i16 = mybir.dt.int16
    AF = mybir.ActivationFunctionType
    ALU = mybir.AluOpType
    AX = mybir.AxisListType

    B, M2, W1, W3 = meta["B"], meta["M2"], meta["W1"], meta["W3"]
    L3, NCHUNK = meta["L3"], meta["NCHUNK"]

    nc = bacc.Bacc(num_devices=NC)
    g_wexp = nc.dram_tensor("w_exp", [128, M1], bf16, kind="ExternalInput")
    g_idx1 = nc.dram_tensor("idx1", [W1, 128, M1], i16, kind="ExternalInput")
    g_idx3 = nc.dram_tensor("idx3", [W3, 128, M2], i16, kind="ExternalInput")
    g_bins = nc.dram_tensor("idx_bins", [128, NCHUNK], i16, kind="ExternalInput")
    g_extf = nc.dram_tensor("extf", [128, COLS], i16, kind="ExternalInput")
    g_sall0 = nc.dram_tensor("s_all0", [128, NODECOLS], bf16, kind="ExternalInput")
    raster = nc.dram_tensor("raster", [steps, 128, COLS], bf16,
                            kind="ExternalOutput")
    cc_in = nc.dram_tensor("cc_in", [128, CPN], bf16)
    cc_out = nc.dram_tensor("cc_out", [NC * 128, CPN], bf16, addr_space="Shared")

    with tile.TileContext(nc) as tc:
        with tc.tile_pool(name="res", bufs=1) as res, \
             tc.tile_pool(name="big", bufs=1) as big, \
             tc.tile_pool(name="idx", bufs=2) as idxp:
            t_wexp = res.tile([128, M1], bf16)
            t_sall = res.tile([128, NODECOLS], bf16)
            t_bins_i = res.tile([128, NCHUNK], i16)
            t_extf = res.tile([128, COLS], i16)
            t_bias = res.tile([128, 1], f32)
            nc.vector.memset(t_bias[:], -THETA)
            nc.sync.dma_start(t_wexp[:], g_wexp[:])
            nc.sync.dma_start(t_sall[:], g_sall0[:])
            nc.sync.dma_start(t_bins_i[:], g_bins[:])
            nc.sync.dma_start(t_extf[:], g_extf[:])

            for t in range(steps):
                # 1. expansion multiply
                t_p1 = big.tile([128, M1], bf16, tag="p1t1")
                s_b = t_sall[:].unsqueeze(1).broadcast_to([128, R, NODECOLS])
                nc.vector.tensor_tensor(
                    t_p1[:].rearrange("p (o j) -> p o j", o=R),
                    t_wexp[:].rearrange("p (o j) -> p o j", o=R),
                    s_b, ALU.mult)

                # 2a. route scatter 1 (P1 -> R1)
                t_r1 = big.tile([128, W1 * WIN], bf16, tag="r1r2")
                for w in range(W1):
                    t_i1 = idxp.tile([128, M1], i16, tag="idx")
                    nc.sync.dma_start(t_i1[:], g_idx1[w])
                    nc.gpsimd.local_scatter(
                        t_r1[:, w * WIN:(w + 1) * WIN], t_p1[:], t_i1[:],
                        channels=128, num_elems=WIN, num_idxs=M1)

                # 2b. 128-block transposes
                t_t1 = big.tile([128, M2], bf16, tag="p1t1")
                nc.vector.transpose(t_t1[:], t_r1[:, :M2])
                t_r2 = big.tile([128, W1 * WIN], bf16, tag="r1r2")
                for a in range(4):
                    for d in range(4):
                        nc.sync.dma_start(
                            t_r2[32 * a:32 * a + 32, :M2]
                            .rearrange("p (b d w) -> p b d w", d=4, w=32)
                            [:, :, d, :],
                            t_t1[32 * d:32 * d + 32, :]
                            .rearrange("p (b a w) -> p b a w", a=4, w=32)
                            [:, :, a, :])

                # 2c. route scatter 2 (R2 -> R3 dst runs)
                t_r3 = big.tile([128, L3], bf16, tag="r3")
                for w in range(W3):
                    t_i3 = idxp.tile([128, M2], i16, tag="idx")
                    nc.sync.dma_start(t_i3[:], g_idx3[w])
                    nc.gpsimd.local_scatter(
                        t_r3[:, w * WIN:min((w + 1) * WIN, L3)], t_r2[:, :M2],
                        t_i3[:], channels=128,
                        num_elems=min(WIN, L3 - w * WIN), num_idxs=M2)

                # 3. segment sums
                t_ch = big.tile([128, NCHUNK], f32, tag="ch")
                nc.vector.tensor_reduce(
                    t_ch[:], t_r3[:].rearrange("p (c e) -> p c e", e=8),
                    axis=AX.X, op=ALU.add)
                t_chb = big.tile([128, NCHUNK], bf16, tag="chb")
                nc.vector.tensor_copy(t_chb[:], t_ch[:])
                t_bins = big.tile([128, MAXRANK * BINS], bf16, tag="bins")
                nc.gpsimd.local_scatter(
                    t_bins[:], t_chb[:], t_bins_i[:],
                    channels=128, num_elems=MAXRANK * BINS, num_idxs=NCHUNK)
                t_sum = big.tile([128, MAXRANK], f32, tag="sum")
                nc.vector.tensor_reduce(
                    t_sum[:], t_bins[:].rearrange("p (r e) -> p r e", e=BINS),
                    axis=AX.X, op=ALU.add)

                # 4. sigmoid + extension fill -> shard
                t_shard = big.tile([128, CPN], bf16, tag="shard")
                nc.scalar.activation(t_shard[:, :COLS], t_sum[:], AF.Sigmoid,
                                     bias=t_bias[:])
                nc.gpsimd.local_scatter(
                    t_shard[:, COLS:CPN], t_shard[:, :COLS], t_extf[:],
                    channels=128, num_elems=EXT, num_idxs=COLS)

                # 5. raster + exchange + s_all rebuild
                nc.sync.dma_start(raster[t], t_shard[:, :COLS])
                nc.sync.dma_start(cc_in[:], t_shard[:])
                nc.gpsimd.collective_compute(
                    "AllGather", ALU.bypass,
                    replica_groups=[list(range(NC))],
                    ins=[cc_in[:]], outs=[cc_out[:]])
                for k in range(NC):
                    nc.sync.dma_start(
                        t_sall[:, CPN * k:CPN * (k + 1)],
                        cc_out[128 * k:128 * (k + 1), :])
    nc.finalize()
    return nc


def _make_runner(nc, in_maps):
    """Compile the SPMD program once; return (run_fn, fetch) where run_fn()
    executes with device-resident inputs (no host->device transfer)."""
    import jax
    import jax.numpy as jnp
    import concourse.mybir as mybir
    from jax.sharding import Mesh, NamedSharding, PartitionSpec as P
    from jax.experimental.shard_map import shard_map
    from concourse import bass2jax

    bass2jax.install_neuronx_cc_hook()

    in_names, out_names, out_avals, zero_shapes = [], [], [], []
    partition_name = (nc.partition_id_tensor.name
                      if nc.partition_id_tensor else None)
    for alloc in nc.m.functions[0].allocations:
        if not isinstance(alloc, mybir.MemoryLocationSet):
            continue
        name = alloc.memorylocations[0].name
        if alloc.kind == "ExternalInput" and name != partition_name:
            in_names.append(name)
        elif alloc.kind == "ExternalOutput":
            out_names.append(name)
            shape = tuple(alloc.tensor_shape)
            dtype = mybir.dt.np(alloc.dtype)
            out_avals.append(jax.core.ShapedArray(shape, dtype))
            zero_shapes.append((shape, dtype))
    n_params = len(in_names)
    n_outs = len(out_names)
    all_in_names = list(in_names) + list(out_names)
    if partition_name is not None:
        all_in_names.append(partition_name)
    donate = tuple(range(n_params, n_params + n_outs))

    def _body(*args):
        operands = list(args)
        if partition_name is not None:
            operands.append(bass2jax.partition_id_tensor())
        return tuple(bass2jax._bass_exec_p.bind(
            *operands,
            out_avals=tuple(out_avals),
            in_names=tuple(all_in_names),
            out_names=tuple(out_names),
            lowering_input_output_aliases=(),
            sim_require_finite=True,
            sim_require_nnan=True,
            nc=nc,
        ))

    devices = jax.devices()[:NC]
    mesh = Mesh(np.asarray(devices), ("core",))
    sharded = jax.jit(
        shard_map(_body, mesh=mesh,
                  in_specs=(P("core"),) * (n_params + n_outs),
                  out_specs=(P("core"),) * n_outs,
                  check_rep=False),
        donate_argnums=donate, keep_unused=True,
    )
    sh = NamedSharding(mesh, P("core"))
    dev_in = [
        jax.device_put(
            np.concatenate([np.asarray(in_maps[c][name])
                            for c in range(NC)], axis=0), sh)
        for name in in_names
    ]
    zero_fns = [
        jax.jit(lambda s=shape, d=dtype: jnp.zeros((NC * s[0],) + s[1:], d),
                out_shardings=sh)
        for shape, dtype in zero_shapes
    ]

    def run_fn():
        zeros = [zf() for zf in zero_fns]
        jax.block_until_ready(zeros)
        import time as _t
        t0 = _t.perf_counter()
        outs = sharded(*dev_in, *zeros)
        jax.block_until_ready(outs)
        return _t.perf_counter() - t0, outs

    def fetch(outs):
        res = []
        for c in range(NC):
            res.append({
                name: np.asarray(outs[i]).reshape(NC, *out_avals[i].shape)[c]
                for i, name in enumerate(out_names)})
        return res

    return run_fn, fetch


def kernel(x, W, edge_index, n_steps, equilibration_steps):
    import time as _t

    global LAST_RUN_NS
    n_steps = int(n_steps)
    equilibration_steps = int(equilibration_steps)
    total = n_steps + equilibration_steps

    t0 = _t.perf_counter()
    meta, per_core = _preprocess(x, W, edge_index)
    print(f"[kernel] preprocess {_t.perf_counter()-t0:.1f}s "
          f"B={meta['B']} W1={meta['W1']} L3={meta['L3']}", flush=True)

    t0 = _t.perf_counter()
    nc = _build(meta, total)
    print(f"[kernel] trace {_t.perf_counter()-t0:.1f}s", flush=True)

    in_maps = []
    for k in range(NC):
        pc = per_core[k]
        in_maps.append({
            "w_exp": np.asarray(pc["w_exp"]),
            "idx1": pc["idx1"],
            "idx3": pc["idx3"],
            "idx_bins": pc["idx_bins"],
            "extf": pc["extf"],
            "s_all0": np.asarray(meta["s_all0"]),
        })

    t0 = _t.perf_counter()
    run_fn, fetch = _make_runner(nc, in_maps)
    dt_cold, outs = run_fn()
    print(f"[kernel] compile+first-exec {_t.perf_counter()-t0:.2f}s",
          flush=True)

    dt, outs = run_fn()
    LAST_RUN_NS = int(dt * 1e9)
    print(f"[kernel] run(warm) {dt:.3f}s ({dt/total*1e3:.2f} ms/step)",
          flush=True)

    results = fetch(outs)
    out = np.empty((n_steps, N_NODES), dtype=np.float32)
    for k in range(NC):
        rk = results[k]["raster"][equilibration_steps:]  # [n, 128, COLS]
        flat = rk.transpose(0, 2, 1).reshape(n_steps, 128 * COLS)
        out[:, NCD * k:NCD * (k + 1)] = flat[:, :NCD].astype(np.float32)
    return out



# revision 2
# speedup vs baseline: 1.7199x; 1.7199x over previous
"""Trainium2 Bass kernel for nn_AbstractModel_70394513981830 (spiking network).

Per step: s' = sigmoid(segment_sum(W * s[src], dst) - 1), 300 sequential steps,
last 200 recorded. 8 NeuronCores, dst-sharded (NC k owns dsts [12500k, +12500)).

V2 layout — coupled-window routing (all scatter scans are one-pass):
- P1 occ-grid [128, R*NODECOLS] (o-major), R=16 occurrence slots per node col.
- R1 pos = 128*b + p_dst, buckets b in [0, Bp), window w = buckets [15w,15w+15)
  (WIN=1920 out elems <= 2047 gpsimd local RAM limit).
- Edge assignment couples b and occ slot: window w's scatter scans ONLY the
  P1 occ-planes O_w = [o_lo[w], o_hi[w]) -> total stage-a scan = M1 (was 6x).
- Stage-c: per (dst, window) sub-runs padded to x8, concatenated per
  (partition, window); window w's scatter scans only its own 1920-col chunk
  of R2 -> total scan = M2 (was 4x). Sub-run chunks re-collected by the bins
  scatter (BINS bins per dst rank), then reduce16 -> sigmoid.
- Indices are SBUF-resident (loaded once); per-window pipeline: scatter1(w)
  || transpose/DMA(w-1) || scatter2(w-1) || reduce8(w-1) keeps gpsimd busy.
- Per-step AllGather of the [128, CPN] shard rebuilds s_all on every core.
"""

import numpy as np

LAST_RUN_NS = 0

N_NODES = 100_000
N_EDGES = 6_400_000
THETA = 1.0
NC = 8
NCD = N_NODES // NC          # 12500
COLS = 98                    # ceil(12500/128)
EXT = 12                     # extension columns per sender block
CPN = COLS + EXT             # 110
NODECOLS = NC * CPN          # 880
R = 16                       # occ slots per node column
M1 = R * NODECOLS            # 14080
BPW = 15                     # buckets per window
WIN = 128 * BPW              # 1920


def _group_rank(keys):
    order = np.argsort(keys, kind="stable")
    sk = keys[order]
    first = np.r_[0, np.flatnonzero(np.diff(sk)) + 1]
    grp_start = np.zeros(len(sk), dtype=np.int64)
    grp_start[first] = first
    np.maximum.accumulate(grp_start, out=grp_start)
    rank_sorted = np.arange(len(sk)) - grp_start
    rank = np.empty(len(sk), dtype=np.int64)
    rank[order] = rank_sorted
    return rank


def _compute_NW(k_r, src, dst):
    q = (src % NCD) % 128
    p_d = (dst % NCD) % 128
    key_qp = (k_r * 128 + q) * 128 + p_d
    B = int(np.bincount(key_qp, minlength=NC * 128 * 128).max())
    NW = -(-B // BPW)
    base = R // NW
    sl = np.full(NW, base, dtype=np.int64)
    sl[:R - base * NW] += 1
    return NW, sl


def _assign_windows(k_r, src, dst, NW, sl):
    """Per-edge (window w, occ slot rank o_rel, on_ext, bucket b)."""
    cap = np.asarray(sl)
    base_seq = np.concatenate([np.arange(NW)] * int(cap.max()))
    base_seq = base_seq[cap[base_seq % NW] > (np.arange(len(base_seq)) // NW)]
    assert len(base_seq) == R
    seq = np.stack([np.roll(base_seq, -r) for r in range(R)])
    o_rel_tab = np.zeros((R, R), dtype=np.int64)
    for rot in range(R):
        cnt = np.zeros(NW, dtype=np.int64)
        for rk in range(R):
            wv = seq[rot, rk]
            o_rel_tab[rot, rk] = cnt[wv]
            cnt[wv] += 1

    q = (src % NCD) % 128
    p_d = (dst % NCD) % 128
    node_key = k_r * N_NODES + src
    rk = _group_rank(node_key)
    assert rk.max() < 2 * R, rk.max()
    rot = (src * 2654435761 % 2**31) % R
    on_ext = rk >= R
    rk_eff = np.where(on_ext, rk - R, rk)
    w = seq[rot, rk_eff]

    key_qp = (k_r * 128 + q) * 128 + p_d
    for _pas in range(6):
        key_qpw = key_qp * NW + w
        rank_b = _group_rank(key_qpw)
        key_o = (node_key * NW + w) * 2 + on_ext
        o_rel = _group_rank(key_o)
        viol = np.flatnonzero((rank_b >= BPW) | (o_rel >= cap[w]))
        if len(viol) == 0:
            break
        bucket_load = np.bincount(key_qpw, minlength=NC * 128 * 128 * NW
                                  ).reshape(-1, NW)
        nk = node_key * NW
        slot_used_b = np.bincount((nk + w)[~on_ext],
                                  minlength=NC * N_NODES * NW).reshape(-1, NW)
        slot_used_e = np.bincount((nk + w)[on_ext],
                                  minlength=NC * N_NODES * NW).reshape(-1, NW)
        for e in viol:
            nkey, qp, we = node_key[e], key_qp[e], int(w[e])
            ext_e = bool(on_ext[e])
            bucket_load[qp][we] -= 1
            (slot_used_e if ext_e else slot_used_b)[nkey][we] -= 1
            moved = False
            for use_ext in [ext_e, not ext_e]:
                used = slot_used_e[nkey] if use_ext else slot_used_b[nkey]
                for w2 in np.argsort(bucket_load[qp], kind="stable"):
                    if bucket_load[qp][w2] >= BPW or used[w2] >= cap[w2]:
                        continue
                    bucket_load[qp][w2] += 1
                    used[w2] += 1
                    w[e] = w2
                    on_ext[e] = use_ext
                    moved = True
                    break
                if moved:
                    break
            if not moved:
                bucket_load[qp][we] += 1
                (slot_used_e if ext_e else slot_used_b)[nkey][we] += 1
    key_qpw = key_qp * NW + w
    rank_b = _group_rank(key_qpw)
    key_o = (node_key * NW + w) * 2 + on_ext
    o_rel = _group_rank(key_o)
    nfail = int(((rank_b >= BPW) | (o_rel >= cap[w])).sum())
    assert nfail == 0, f"window assignment failures: {nfail}"
    b = w * BPW + rank_b
    return dict(w=w, o_rel=o_rel, on_ext=on_ext, b=b, q=q, p_d=p_d)


def _preprocess(x, W, edge_index):
    import ml_dtypes
    src = np.asarray(edge_index[0], dtype=np.int64)
    dst = np.asarray(edge_index[1], dtype=np.int64)
    Wf = np.asarray(W, dtype=np.float32)
    s0 = np.asarray(x, dtype=np.float32).reshape(-1)

    k_r = dst // NCD
    NW, sl = _compute_NW(k_r, src, dst)
    res = _assign_windows(k_r, src, dst, NW, sl)
    w_e, o_rel, on_ext, b = res["w"], res["o_rel"], res["on_ext"], res["b"]
    q, p_d = res["q"], res["p_d"]
    Bp = BPW * NW
    o_hi = np.cumsum(sl)
    o_lo = o_hi - sl
    o_e = o_lo[w_e] + o_rel

    ext_nodes = np.unique(src[on_ext])
    nq = (ext_nodes % NCD) % 128
    nk = ext_nodes // NCD
    ext_rank = _group_rank(nk * 128 + nq)
    assert ext_rank.max() < EXT, f"EXT overflow: {ext_rank.max() + 1}"
    ext_slot = np.full(N_NODES, -1, dtype=np.int64)
    ext_slot[ext_nodes] = ext_rank
    ext_col = np.full(N_NODES, -1, dtype=np.int64)
    ext_col[ext_nodes] = CPN * nk + COLS + ext_rank

    base_col = CPN * (src // NCD) + (src % NCD) // 128
    j_e = np.where(on_ext, ext_col[src], base_col)
    assert (j_e >= 0).all()
    m1 = o_e * NODECOLS + j_e
    r_d = (dst % NCD) // 128

    key_pwr = ((k_r * 128 + p_d) * NW + w_e) * COLS + r_d
    deg = np.bincount(key_pwr, minlength=NC * 128 * NW * COLS
                      ).reshape(NC, 128, NW, COLS)
    pad = (deg + 7) // 8 * 8
    sum_pw = pad.sum(axis=3)
    L3w = -(-sum_pw.max(axis=(0, 1)) // 16) * 16
    assert (L3w <= 2046).all(), L3w
    L3off = np.concatenate([[0], np.cumsum(L3w)])
    L3 = int(L3off[-1])
    NCHUNK = L3 // 8
    starts = np.zeros_like(pad)
    starts[:, :, :, 1:] = np.cumsum(pad, axis=3)[:, :, :, :-1]
    BINS = int((pad // 8).sum(axis=2).max())
    NHALF = 1 if COLS * BINS <= 2046 else 2
    HR = (COLS + NHALF - 1) // NHALF         # ranks per half

    within = _group_rank(key_pwr)
    slot3 = L3off[w_e] + starts[k_r, p_d, w_e, r_d] + within
    m2c = 128 * (b - BPW * w_e) + q

    chunk_sz = sl * NODECOLS
    per_core = []
    for k in range(NC):
        sel = np.flatnonzero(k_r == k)
        eq, ep, eb, ew = q[sel], p_d[sel], b[sel], w_e[sel]
        em1, eslot3 = m1[sel], slot3[sel]
        idx1 = np.full((128, M1), -1, dtype=np.int16)
        r1off = 128 * (eb - BPW * ew) + ep
        idx1[eq, em1] = r1off.astype(np.int16)
        idx3 = np.full((128, NW * WIN), -1, dtype=np.int16)
        idx3[ep, ew * WIN + 128 * (eb - BPW * ew) + eq] = (
            eslot3 - L3off[ew]).astype(np.int16)
        idx_bins = np.full((128, NHALF * NCHUNK), -1, dtype=np.int16)
        padk = pad[k]
        startsk = starts[k]
        binbase = np.zeros_like(padk)
        binbase[:, 1:, :] = np.cumsum(padk // 8, axis=1)[:, :-1, :]
        for p in range(128):
            for wv in range(NW):
                nz = np.flatnonzero(padk[p, wv])
                for rr in nz:
                    nch = padk[p, wv, rr] // 8
                    c0 = (L3off[wv] + startsk[p, wv, rr]) // 8
                    half = rr // HR
                    rb = rr - half * HR
                    bb = rb * BINS + binbase[p, wv, rr]
                    idx_bins[p, half * NCHUNK + c0:half * NCHUNK + c0 + nch] \
                        = np.arange(bb, bb + nch, dtype=np.int16)
        w_exp = np.zeros((128, M1), dtype=ml_dtypes.bfloat16)
        w_exp[eq, em1] = Wf[sel].astype(ml_dtypes.bfloat16)
        extf = np.full((128, COLS), -1, dtype=np.int16)
        own = np.arange(NCD * k, NCD * (k + 1))
        sown = ext_slot[own]
        d_own = np.flatnonzero(sown >= 0)
        extf[d_own % 128, d_own // 128] = sown[d_own].astype(np.int16)
        per_core.append(dict(w_exp=w_exp, idx1=idx1, idx3=idx3,
                             idx_bins=idx_bins, extf=extf))

    s_pad = np.zeros((128, NODECOLS), dtype=np.float32)
    n = np.arange(N_NODES)
    s_pad[(n % NCD) % 128, CPN * (n // NCD) + (n % NCD) // 128] = s0
    ei = np.flatnonzero(ext_col >= 0)
    s_pad[(ei % NCD) % 128, ext_col[ei]] = s0[ei]
    s_all0 = s_pad.astype(ml_dtypes.bfloat16)

    meta = dict(NW=NW, Bp=Bp, sl=sl, o_lo=o_lo, chunk_sz=chunk_sz,
                L3w=L3w, L3off=L3off, L3=L3, NCHUNK=NCHUNK, BINS=BINS,
                NHALF=NHALF, HR=HR, s_all0=s_all0)
    return meta, per_core


def _build(meta, steps):
    """Build the unrolled bass program (shared by all 8 cores)."""
    import concourse.bass as bass
    import concourse.bacc as bacc
    import concourse.mybir as mybir
    import concourse.tile as tile

    f32 = mybir.dt.float32
    bf16 = mybir.dt.bfloat16
    i16 = mybir.dt.int16
    AF = mybir.ActivationFunctionType
    ALU = mybir.AluOpType
    AX = mybir.AxisListType

    NW, sl, o_lo = meta["NW"], meta["sl"], meta["o_lo"]
    chunk_sz = meta["chunk_sz"]
    L3w, L3off, L3 = meta["L3w"], meta["L3off"], meta["L3"]
    NCHUNK, BINS, NHALF, HR = (meta["NCHUNK"], meta["BINS"], meta["NHALF"],
                               meta["HR"])

    nc = bacc.Bacc(num_devices=NC)
    g_wexp = nc.dram_tensor("w_exp", [128, M1], bf16, kind="ExternalInput")
    g_idx1 = nc.dram_tensor("idx1", [128, M1], i16, kind="ExternalInput")
    g_idx3 = nc.dram_tensor("idx3", [128, NW * WIN], i16, kind="ExternalInput")
    g_bins = nc.dram_tensor("idx_bins", [128, NHALF * NCHUNK], i16,
                            kind="ExternalInput")
    g_extf = nc.dram_tensor("extf", [128, COLS], i16, kind="ExternalInput")
    g_sall0 = nc.dram_tensor("s_all0", [128, NODECOLS], bf16,
                             kind="ExternalInput")
    raster = nc.dram_tensor("raster", [steps, 128, COLS], bf16,
                            kind="ExternalOutput")
    cc_in = nc.dram_tensor("cc_in", [128, CPN], bf16)
    cc_out = nc.dram_tensor("cc_out", [NC * 128, CPN], bf16,
                            addr_space="Shared")

    with tile.TileContext(nc) as tc:
        with tc.tile_pool(name="res", bufs=1) as res, \
             tc.tile_pool(name="pw", bufs=2) as pw, \
             tc.tile_pool(name="pr3", bufs=2) as pr3, \
             tc.tile_pool(name="tail", bufs=2) as tail:
            t_wexp = res.tile([128, M1], bf16)
            t_idx1 = res.tile([128, M1], i16)
            t_idx3 = res.tile([128, NW * WIN], i16)
            t_bins_i = res.tile([128, NHALF * NCHUNK], i16)
            t_extf = res.tile([128, COLS], i16)
            t_sall = res.tile([128, NODECOLS], bf16)
            t_bias = res.tile([128, 1], f32)
            nc.vector.memset(t_bias[:], -THETA)
            nc.sync.dma_start(t_wexp[:], g_wexp[:])
            nc.sync.dma_start(t_idx1[:], g_idx1[:])
            nc.sync.dma_start(t_idx3[:], g_idx3[:])
            nc.sync.dma_start(t_bins_i[:], g_bins[:])
            nc.sync.dma_start(t_extf[:], g_extf[:])
            nc.sync.dma_start(t_sall[:], g_sall0[:])

            for t in range(steps):
                t_ch = tail.tile([128, NCHUNK], f32, tag="ch")
                t_chb = tail.tile([128, NCHUNK], bf16, tag="chb")

                def stage2(w, t_r2):
                    lw = int(L3w[w])
                    t_r3 = pr3.tile([128, lw], bf16, tag="r3")
                    nc.gpsimd.local_scatter(
                        t_r3[:], t_r2[:],
                        t_idx3[:, w * WIN:(w + 1) * WIN],
                        channels=128, num_elems=lw, num_idxs=WIN)
                    co, cw = int(L3off[w]) // 8, lw // 8
                    nc.vector.tensor_reduce(
                        t_ch[:, co:co + cw],
                        t_r3[:].rearrange("p (c e) -> p c e", e=8),
                        axis=AX.X, op=ALU.add)
                    nc.vector.tensor_copy(t_chb[:, co:co + cw],
                                          t_ch[:, co:co + cw])

                prev = None
                for w in range(NW):
                    csz = int(chunk_sz[w])
                    slw = int(sl[w])
                    off = int(o_lo[w]) * NODECOLS
                    t_p1 = pw.tile([128, int(chunk_sz.max())], bf16, tag="p1")
                    s_b = t_sall[:].unsqueeze(1).broadcast_to(
                        [128, slw, NODECOLS])
                    nc.vector.tensor_tensor(
                        t_p1[:, :csz].rearrange("p (o j) -> p o j", o=slw),
                        t_wexp[:, off:off + csz].rearrange(
                            "p (o j) -> p o j", o=slw),
                        s_b, ALU.mult)
                    t_r1 = pw.tile([128, WIN], bf16, tag="r1")
                    nc.gpsimd.local_scatter(
                        t_r1[:], t_p1[:, :csz], t_idx1[:, off:off + csz],
                        channels=128, num_elems=WIN, num_idxs=csz)
                    t_t1 = pw.tile([128, WIN], bf16, tag="t1")
                    nc.vector.transpose(t_t1[:], t_r1[:])
                    t_r2 = pw.tile([128, WIN], bf16, tag="r2")
                    for a in range(4):
                        for d in range(4):
                            nc.sync.dma_start(
                                t_r2[32 * a:32 * a + 32, :]
                                .rearrange("p (b d x) -> p b d x", d=4, x=32)
                                [:, :, d, :],
                                t_t1[32 * d:32 * d + 32, :]
                                .rearrange("p (b a x) -> p b a x", a=4, x=32)
                                [:, :, a, :])
                    if prev is not None:
                        stage2(*prev)
                    prev = (w, t_r2)
                stage2(*prev)

                t_bins = tail.tile([128, COLS * BINS], bf16, tag="bins")
                for h in range(NHALF):
                    nh = COLS - h * HR if h == NHALF - 1 else HR
                    nc.gpsimd.local_scatter(
                        t_bins[:, h * HR * BINS:(h * HR + nh) * BINS],
                        t_chb[:],
                        t_bins_i[:, h * NCHUNK:(h + 1) * NCHUNK],
                        channels=128, num_elems=nh * BINS, num_idxs=NCHUNK)
                t_sum = tail.tile([128, COLS], f32, tag="sum")
                nc.vector.tensor_reduce(
                    t_sum[:], t_bins[:].rearrange("p (r e) -> p r e", e=BINS),
                    axis=AX.X, op=ALU.add)
                t_shard = tail.tile([128, CPN], bf16, tag="shard")
                nc.scalar.activation(t_shard[:, :COLS], t_sum[:], AF.Sigmoid,
                                     bias=t_bias[:])
                nc.gpsimd.local_scatter(
                    t_shard[:, COLS:CPN], t_shard[:, :COLS], t_extf[:],
                    channels=128, num_elems=EXT, num_idxs=COLS)

                nc.sync.dma_start(raster[t], t_shard[:, :COLS])
                nc.sync.dma_start(cc_in[:], t_shard[:])
                nc.gpsimd.collective_compute(
                    "AllGather", ALU.bypass,
                    replica_groups=[list(range(NC))],
                    ins=[cc_in[:]], outs=[cc_out[:]])
                for k in range(NC):
                    nc.sync.dma_start(
                        t_sall[:, CPN * k:CPN * (k + 1)],
                        cc_out[128 * k:128 * (k + 1), :])
    nc.finalize()
    return nc


def _make_runner(nc, in_maps):
    """Compile the SPMD program once; return (run_fn, fetch)."""
    import jax
    import jax.numpy as jnp
    import concourse.mybir as mybir
    from jax.sharding import Mesh, NamedSharding, PartitionSpec as P
    from jax.experimental.shard_map import shard_map
    from concourse import bass2jax

    bass2jax.install_neuronx_cc_hook()

    in_names, out_names, out_avals, zero_shapes = [], [], [], []
    partition_name = (nc.partition_id_tensor.name
                      if nc.partition_id_tensor else None)
    for alloc in nc.m.functions[0].allocations:
        if not isinstance(alloc, mybir.MemoryLocationSet):
            continue
        name = alloc.memorylocations[0].name
        if alloc.kind == "ExternalInput" and name != partition_name:
            in_names.append(name)
        elif alloc.kind == "ExternalOutput":
            out_names.append(name)
            shape = tuple(alloc.tensor_shape)
            dtype = mybir.dt.np(alloc.dtype)
            out_avals.append(jax.core.ShapedArray(shape, dtype))
            zero_shapes.append((shape, dtype))
    n_params = len(in_names)
    n_outs = len(out_names)
    all_in_names = list(in_names) + list(out_names)
    if partition_name is not None:
        all_in_names.append(partition_name)
    donate = tuple(range(n_params, n_params + n_outs))

    def _body(*args):
        operands = list(args)
        if partition_name is not None:
            operands.append(bass2jax.partition_id_tensor())
        return tuple(bass2jax._bass_exec_p.bind(
            *operands,
            out_avals=tuple(out_avals),
            in_names=tuple(all_in_names),
            out_names=tuple(out_names),
            lowering_input_output_aliases=(),
            sim_require_finite=True,
            sim_require_nnan=True,
            nc=nc,
        ))

    devices = jax.devices()[:NC]
    mesh = Mesh(np.asarray(devices), ("core",))
    sharded = jax.jit(
        shard_map(_body, mesh=mesh,
                  in_specs=(P("core"),) * (n_params + n_outs),
                  out_specs=(P("core"),) * n_outs,
                  check_rep=False),
        donate_argnums=donate, keep_unused=True,
    )
    sh = NamedSharding(mesh, P("core"))
    dev_in = [
        jax.device_put(
            np.concatenate([np.asarray(in_maps[c][name])
                            for c in range(NC)], axis=0), sh)
        for name in in_names
    ]
    zero_fns = [
        jax.jit(lambda s=shape, d=dtype: jnp.zeros((NC * s[0],) + s[1:], d),
                out_shardings=sh)
        for shape, dtype in zero_shapes
    ]

    def run_fn():
        zeros = [zf() for zf in zero_fns]
        jax.block_until_ready(zeros)
        import time as _t
        t0 = _t.perf_counter()
        outs = sharded(*dev_in, *zeros)
        jax.block_until_ready(outs)
        return _t.perf_counter() - t0, outs

    def fetch(outs):
        res = []
        for c in range(NC):
            res.append({
                name: np.asarray(outs[i]).reshape(NC, *out_avals[i].shape)[c]
                for i, name in enumerate(out_names)})
        return res

    return run_fn, fetch


def kernel(x, W, edge_index, n_steps, equilibration_steps):
    import time as _t

    global LAST_RUN_NS
    n_steps = int(n_steps)
    equilibration_steps = int(equilibration_steps)
    total = n_steps + equilibration_steps

    t0 = _t.perf_counter()
    meta, per_core = _preprocess(x, W, edge_index)
    print(f"[kernel] preprocess {_t.perf_counter()-t0:.1f}s "
          f"NW={meta['NW']} L3={meta['L3']} BINS={meta['BINS']}", flush=True)

    t0 = _t.perf_counter()
    nc = _build(meta, total)
    print(f"[kernel] trace {_t.perf_counter()-t0:.1f}s", flush=True)

    in_maps = []
    for k in range(NC):
        pc = per_core[k]
        in_maps.append({
            "w_exp": np.asarray(pc["w_exp"]),
            "idx1": pc["idx1"],
            "idx3": pc["idx3"],
            "idx_bins": pc["idx_bins"],
            "extf": pc["extf"],
            "s_all0": np.asarray(meta["s_all0"]),
        })

    t0 = _t.perf_counter()
    run_fn, fetch = _make_runner(nc, in_maps)
    dt_cold, outs = run_fn()
    print(f"[kernel] compile+first-exec {_t.perf_counter()-t0:.2f}s",
          flush=True)

    dt, outs = run_fn()
    LAST_RUN_NS = int(dt * 1e9)
    print(f"[kernel] run(warm) {dt:.3f}s ({dt/total*1e3:.2f} ms/step)",
          flush=True)

    results = fetch(outs)
    out = np.empty((n_steps, N_NODES), dtype=np.float32)
    for k in range(NC):
        rk = results[k]["raster"][equilibration_steps:]
        flat = rk.transpose(0, 2, 1).reshape(n_steps, 128 * COLS)
        out[:, NCD * k:NCD * (k + 1)] = flat[:, :NCD].astype(np.float32)
    return out
